# revision 11
# baseline (speedup 1.0000x reference)
"""Trainium2 Bass kernel for nn_CortexNetwork (dense_cnn, memory-bound).

Reference computation:
    patches[c,i,j,u,v] = x[c, rx[i]+u, ry[j]+v]
    aff[i,j] = sum_{c,u,v} patches * Wa
    exc[i,j] = sum_c prev[c,i,j] * sum_{x,y} We[c,i,j,x,y]   (inh likewise, Wi)
    out      = broadcast_c(relu(aff + 0.9*exc - 0.9*inh))

Strategy: tensor-parallel over the 36x36=1296 grid units, 162 units per
core (padded to 168 = 21 groups of 8 so every DMA covers the full 128
partitions; partition = c*8+s).  The output depends on the lateral
weights only through D = We - Wi (same prev multiplier, same gamma), so
the host ships D instead of both tensors, quantized to int8 with
per-(c,unit)-row absmax scales that fold into the per-partition
possb = 0.9*prev*sD; Wa and the gathered patches ship as bf16 so the
fused afferent multiply runs in the DVE 2x packed mode.  Exact offline
rel-err of this scheme on the true inputs is 0.0070 vs the 2e-2 gate.
Per unit the device streams 1296B (D int8) + 2304B (Wa|patch bf16)
= 3600B -> 9.7MB/core, moved by 6 large column-slice DMAs from one
DRAM blob (few tensors keep the NEFF preamble TENSOR_LOADs short).

Per DMA batch (gcnt groups): one 3D bf16 multiply and one batched
tensor_reduce on DVE produce the afferent partials; the lateral reduce
is one scaled reduction per group, interleaved over ScalarE
(activation Copy + scale + accum_out, 17 groups) and DVE
(tensor_scalar mult + accum_out, 4 groups) so both engines drain
concurrently with the DMA stream.  Channel sums are 0/1-selector
matmuls on the idle PE, then relu.
"""

import numpy as np
import ml_dtypes

import concourse.bass as bass
import concourse.bacc as bacc
import concourse.mybir as mybir
from concourse import tile
from concourse.bass_utils import run_bass_kernel_spmd

N_CORES = 8
C = 16
GX = GY = 36
RF = 24
IMG = 64
GAMMA = 0.9

UNITS = GX * GY                  # 1296
PER_CORE = UNITS // N_CORES      # 162
S = 8                            # units per group (partition dim C*S=128)
NG = 21                          # groups per core (168 units, 6 padded)
PAD = NG * S                     # 168
FW = GX * GY                     # lateral cols per unit: 1296
FA = RF * RF                     # afferent cols per unit: 576
UB = FW + 4 * FA                 # bytes per unit: 1296 + 2304 = 3600
UH = UB // 2                     # bf16 elements per unit view: 1800

DMA_G = [2, 4, 5, 5, 5]
DMA_START = np.concatenate([[0], np.cumsum(DMA_G)]).tolist()
DVE_D_GROUPS = (4, 9, 14, 19)    # lateral reduce on DVE; rest on ScalarE

_PROGRAM_CACHE = {}


def _build_program():
    f32 = mybir.dt.float32
    i8 = mybir.dt.int8
    bf16 = mybir.dt.bfloat16
    AL = mybir.AluOpType
    AF = mybir.ActivationFunctionType

    nc = bacc.Bacc(
        "TRN2", target_bir_lowering=False, debug=False, num_devices=N_CORES
    )
    blob_d = nc.dram_tensor("blob", [128, NG * UB], i8, kind="ExternalInput").ap()
    # consts: possb[0:21] | sap[21:42] | sel[42:50]
    consts_d = nc.dram_tensor("consts", [128, 2 * NG + S], f32, kind="ExternalInput").ap()
    out_d = nc.dram_tensor("out", [S, NG], f32, kind="ExternalOutput").ap()

    with tile.TileContext(nc) as tc:
        with (
            tc.tile_pool(name="w", bufs=1) as wp,
            tc.tile_pool(name="cst", bufs=1) as cp,
            tc.tile_pool(name="junk", bufs=1) as jp,
            tc.tile_pool(name="fin", bufs=1) as fp,
            tc.tile_pool(name="ps", bufs=1, space="PSUM") as pp,
        ):
            consts = cp.tile([128, 2 * NG + S], f32, tag="consts")
            plat = cp.tile([128, NG], f32, tag="plat")
            paffr = cp.tile([128, NG], f32, tag="paffr")
            warm = cp.tile([128, 1], f32, tag="warm")
            nc.sync.dma_start(consts[:], consts_d[:])

            wtiles = []
            for i, gcnt in enumerate(DMA_G):
                g0 = DMA_START[i]
                w = wp.tile([128, gcnt, UB], i8, tag=f"w{i}", name=f"w{i}")
                q = nc.scalar if i == 0 else nc.sync
                q.dma_start(w[:], blob_d[:, g0 * UB:(g0 + gcnt) * UB])
                wtiles.append(w)

            # warm the ACT spline table before the stream lands so the
            # first real activation doesn't pay the table load
            nc.scalar.activation(warm[:], consts[:, 0:1], AF.Copy)

            js = jp.tile([128, FW], bf16, tag="js")
            jv = jp.tile([128, FW], bf16, tag="jv")

            for i, gcnt in enumerate(DMA_G):
                g0 = DMA_START[i]
                w = wtiles[i]
                wb = w[:].bitcast(bf16)          # [128, gcnt, UH]
                jprod = jp.tile([128, gcnt, FA], bf16, tag=f"jp{i}", name=f"jprod{i}")
                nc.vector.tensor_mul(
                    jprod[:], wb[:, :, 648:648 + FA], wb[:, :, 648 + FA:UH]
                )
                nc.vector.tensor_reduce(
                    paffr[:, g0:g0 + gcnt], jprod[:],
                    axis=mybir.AxisListType.X, op=AL.add,
                )
                for gl in range(gcnt):
                    g = g0 + gl
                    dv = w[:, gl, 0:FW]
                    possb_col = consts[:, g:g + 1]
                    if g in DVE_D_GROUPS:
                        nc.vector.tensor_scalar(
                            jv[:], dv, possb_col, 0.0, AL.mult, AL.add,
                            accum_out=plat[:, g:g + 1],
                        )
                    else:
                        nc.scalar.activation(
                            js[:], dv, AF.Copy, scale=possb_col,
                            accum_out=plat[:, g:g + 1],
                        )

            # afferent row scales, then channel sums on PE
            paf2 = fp.tile([128, NG], f32, tag="paf2")
            nc.vector.tensor_mul(paf2[:], paffr[:], consts[:, NG:2 * NG])
            sel = consts[:, 2 * NG:2 * NG + S]
            psum = pp.tile([S, NG], f32, tag="ps")
            nc.tensor.matmul(psum[:], sel, plat[:], start=True, stop=False)
            nc.tensor.matmul(psum[:], sel, paf2[:], start=False, stop=True)

            res = fp.tile([S, NG], f32, tag="res")
            nc.vector.tensor_scalar_max(res[:], psum[:], 0.0)
            nc.sync.dma_start(out_d[:], res[:])

    nc.compile()
    return nc


def _get_program():
    if "nc" not in _PROGRAM_CACHE:
        _PROGRAM_CACHE["nc"] = _build_program()
    return _PROGRAM_CACHE["nc"]


def _quant_row(a):
    """Per-(c,row) symmetric int8 quantization of [C, N, K] -> int8, scale[C,N]."""
    s = np.abs(a).max(axis=2) / 127.0
    s = np.maximum(s, 1e-30)
    q = np.clip(np.round(a / s[:, :, None]), -127, 127).astype(np.int8)
    return q, s


def _prep_in_maps(inputs):
    x = np.asarray(inputs["x"], dtype=np.float32)
    prev = np.asarray(inputs["prev_activity"], dtype=np.float32).reshape(C, UNITS)
    wa = np.asarray(inputs["afferent_weights"], dtype=np.float32).reshape(C, UNITS, FA)
    we = np.asarray(inputs["ex_lateral_weights"], dtype=np.float32).reshape(C, UNITS, FW)
    wi = np.asarray(inputs["in_lateral_weights"], dtype=np.float32).reshape(C, UNITS, FW)
    rx = np.asarray(inputs["rx"]).astype(np.int64)
    ry = np.asarray(inputs["ry"]).astype(np.int64)

    u = np.arange(RF)
    ix = rx[:, None] + u                     # [GX, RF]
    iy = ry[:, None] + u                     # [GY, RF]
    px = x[:, ix, :]                         # [C, GX, RF, IMG]
    patches = px[:, :, :, iy]                # [C, GX, RF, GY, RF]
    patches = np.ascontiguousarray(patches.transpose(0, 1, 3, 2, 4))
    patches = patches.reshape(C, UNITS, FA)

    qd, sd = _quant_row(we - wi)
    wab = wa.astype(ml_dtypes.bfloat16).view(np.int8).reshape(C, UNITS, 2 * FA)
    pab = patches.astype(ml_dtypes.bfloat16).view(np.int8).reshape(C, UNITS, 2 * FA)
    blk = np.concatenate([qd, wab, pab], axis=2)     # [C, UNITS, UB] bytes
    possb_all = GAMMA * prev * sd                    # [C, UNITS]

    selm = (np.arange(128)[:, None] % S == np.arange(S)[None, :]).astype(np.float32)

    in_maps = []
    for k in range(N_CORES):
        n0 = k * PER_CORE
        b = np.zeros((C, PAD, UB), np.int8)
        b[:, :PER_CORE] = blk[:, n0:n0 + PER_CORE]
        pb = np.zeros((C, PAD), np.float32)
        pb[:, :PER_CORE] = possb_all[:, n0:n0 + PER_CORE]

        blob = b.reshape(C, NG, S, UB).transpose(0, 2, 1, 3).reshape(128, NG * UB)
        cst = np.zeros((128, 2 * NG + S), np.float32)
        cst[:, 0:NG] = pb.reshape(C, NG, S).transpose(0, 2, 1).reshape(128, NG)
        cst[:, NG:2 * NG] = 1.0                      # bf16 afferent: unit scale
        cst[:, 2 * NG:] = selm
        in_maps.append({
            "blob": np.ascontiguousarray(blob),
            "consts": cst,
        })
    return in_maps


def _assemble_output(results):
    act = np.empty(UNITS, np.float32)
    for k in range(N_CORES):
        o = np.asarray(results[k]["out"])            # [S, NG]
        loc = o.T.reshape(PAD)                       # unit n_local = 8g + s
        act[k * PER_CORE:(k + 1) * PER_CORE] = loc[:PER_CORE]
    out = np.broadcast_to(act.reshape(1, GX, GY), (C, GX, GY))
    return np.ascontiguousarray(out, dtype=np.float32)


def kernel(**inputs):
    nc = _get_program()
    in_maps = _prep_in_maps(inputs)
    res = run_bass_kernel_spmd(nc, in_maps, core_ids=list(range(N_CORES)))
    return _assemble_output(res.results)


# revision 13
# speedup vs baseline: 1.0537x; 1.0537x over previous
"""Trainium2 Bass kernel for nn_CortexNetwork (dense_cnn, memory-bound).

Reference computation:
    patches[c,i,j,u,v] = x[c, rx[i]+u, ry[j]+v]
    aff[i,j] = sum_{c,u,v} patches * Wa
    exc[i,j] = sum_c prev[c,i,j] * sum_{x,y} We[c,i,j,x,y]   (inh likewise, Wi)
    out      = broadcast_c(relu(aff + 0.9*exc - 0.9*inh))

Strategy: tensor-parallel over the 36x36=1296 grid units, 162 units per
core (padded to 168 = 21 groups of 8 so every DMA covers the full 128
partitions; partition = c*8+s).  The output depends on the lateral
weights only through D = We - Wi (same prev multiplier, same gamma), so
the host ships D instead of both tensors, quantized to int8 with
per-(c,unit)-row absmax scales that fold into the per-partition
possb = 0.9*prev*sD; Wa and the gathered patches ship as bf16 so the
fused afferent multiply runs in the DVE 2x packed mode.  Exact offline
rel-err of this scheme on the true inputs is 0.0070 vs the 2e-2 gate.
Per unit the device streams 1296B (D int8) + 2304B (Wa|patch bf16)
= 3600B -> 9.7MB/core, moved by 6 large column-slice DMAs from one
DRAM blob (few tensors keep the NEFF preamble TENSOR_LOADs short).

Per DMA batch (gcnt groups): one 3D bf16 multiply and one batched
tensor_reduce on DVE produce the afferent partials; the lateral reduce
is one scaled reduction per group, interleaved over ScalarE
(activation Copy + scale + accum_out, 17 groups) and DVE
(tensor_scalar mult + accum_out, 4 groups) so both engines drain
concurrently with the DMA stream.  Channel sums are 0/1-selector
matmuls on the idle PE, then relu.
"""

import numpy as np
import ml_dtypes

import concourse.bass as bass
import concourse.bacc as bacc
import concourse.mybir as mybir
from concourse import tile
from concourse.bass_utils import run_bass_kernel_spmd

N_CORES = 8
C = 16
GX = GY = 36
RF = 24
IMG = 64
GAMMA = 0.9

UNITS = GX * GY                  # 1296
PER_CORE = UNITS // N_CORES      # 162
S = 8                            # units per group (partition dim C*S=128)
NG = 21                          # groups per core (168 units, 6 padded)
PAD = NG * S                     # 168
FW = GX * GY                     # lateral cols per unit: 1296
FA = RF * RF                     # afferent cols per unit: 576
UB = FW + 4 * FA                 # bytes per unit: 1296 + 2304 = 3600
UH = UB // 2                     # bf16 elements per unit view: 1800

DMA_G = [2, 4, 4, 4, 4, 3]
DMA_START = np.concatenate([[0], np.cumsum(DMA_G)]).tolist()
DVE_D_GROUPS = (4, 9, 14, 19)    # lateral reduce on DVE; rest on ScalarE

_PROGRAM_CACHE = {}


def _build_program():
    f32 = mybir.dt.float32
    i8 = mybir.dt.int8
    bf16 = mybir.dt.bfloat16
    AL = mybir.AluOpType
    AF = mybir.ActivationFunctionType

    nc = bacc.Bacc(
        "TRN2", target_bir_lowering=False, debug=False, num_devices=N_CORES
    )
    blob_d = nc.dram_tensor("blob", [128, NG * UB], i8, kind="ExternalInput").ap()
    # consts: possb[0:21] | sap[21:42] | sel[42:50]
    consts_d = nc.dram_tensor("consts", [128, 2 * NG + S], f32, kind="ExternalInput").ap()
    out_d = nc.dram_tensor("out", [S, NG], f32, kind="ExternalOutput").ap()

    with tile.TileContext(nc) as tc:
        with (
            tc.tile_pool(name="w", bufs=1) as wp,
            tc.tile_pool(name="cst", bufs=1) as cp,
            tc.tile_pool(name="junk", bufs=1) as jp,
            tc.tile_pool(name="fin", bufs=1) as fp,
            tc.tile_pool(name="ps", bufs=1, space="PSUM") as pp,
        ):
            consts = cp.tile([128, 2 * NG + S], f32, tag="consts")
            plat = cp.tile([128, NG], f32, tag="plat")
            paffr = cp.tile([128, NG], f32, tag="paffr")
            warm = cp.tile([128, 1], f32, tag="warm")
            nc.sync.dma_start(consts[:], consts_d[:])

            wtiles = []
            for i, gcnt in enumerate(DMA_G):
                g0 = DMA_START[i]
                w = wp.tile([128, gcnt, UB], i8, tag=f"w{i}", name=f"w{i}")
                nc.sync.dma_start(w[:], blob_d[:, g0 * UB:(g0 + gcnt) * UB])
                wtiles.append(w)

            # warm the ACT spline table before the stream lands so the
            # first real activation doesn't pay the table load
            nc.scalar.activation(warm[:], consts[:, 0:1], AF.Copy)

            js = jp.tile([128, FW], bf16, tag="js")
            jv = jp.tile([128, FW], bf16, tag="jv")

            for i, gcnt in enumerate(DMA_G):
                g0 = DMA_START[i]
                w = wtiles[i]
                wb = w[:].bitcast(bf16)          # [128, gcnt, UH]
                jprod = jp.tile([128, gcnt, FA], bf16, tag=f"jp{i}", name=f"jprod{i}")
                nc.vector.tensor_mul(
                    jprod[:], wb[:, :, 648:648 + FA], wb[:, :, 648 + FA:UH]
                )
                nc.vector.tensor_reduce(
                    paffr[:, g0:g0 + gcnt], jprod[:],
                    axis=mybir.AxisListType.X, op=AL.add,
                )
                for gl in range(gcnt):
                    g = g0 + gl
                    dv = w[:, gl, 0:FW]
                    possb_col = consts[:, g:g + 1]
                    if g in DVE_D_GROUPS:
                        nc.vector.tensor_scalar(
                            jv[:], dv, possb_col, 0.0, AL.mult, AL.add,
                            accum_out=plat[:, g:g + 1],
                        )
                    else:
                        nc.scalar.activation(
                            js[:], dv, AF.Copy, scale=possb_col,
                            accum_out=plat[:, g:g + 1],
                        )

            # afferent row scales, then channel sums on PE
            paf2 = fp.tile([128, NG], f32, tag="paf2")
            nc.vector.tensor_mul(paf2[:], paffr[:], consts[:, NG:2 * NG])
            sel = consts[:, 2 * NG:2 * NG + S]
            psum = pp.tile([S, NG], f32, tag="ps")
            nc.tensor.matmul(psum[:], sel, plat[:], start=True, stop=False)
            nc.tensor.matmul(psum[:], sel, paf2[:], start=False, stop=True)

            res = fp.tile([S, NG], f32, tag="res")
            nc.vector.tensor_scalar_max(res[:], psum[:], 0.0)
            nc.sync.dma_start(out_d[:], res[:])

    nc.compile()
    return nc


def _get_program():
    if "nc" not in _PROGRAM_CACHE:
        _PROGRAM_CACHE["nc"] = _build_program()
    return _PROGRAM_CACHE["nc"]


def _quant_row(a):
    """Per-(c,row) symmetric int8 quantization of [C, N, K] -> int8, scale[C,N]."""
    s = np.abs(a).max(axis=2) / 127.0
    s = np.maximum(s, 1e-30)
    q = np.clip(np.round(a / s[:, :, None]), -127, 127).astype(np.int8)
    return q, s


def _prep_in_maps(inputs):
    x = np.asarray(inputs["x"], dtype=np.float32)
    prev = np.asarray(inputs["prev_activity"], dtype=np.float32).reshape(C, UNITS)
    wa = np.asarray(inputs["afferent_weights"], dtype=np.float32).reshape(C, UNITS, FA)
    we = np.asarray(inputs["ex_lateral_weights"], dtype=np.float32).reshape(C, UNITS, FW)
    wi = np.asarray(inputs["in_lateral_weights"], dtype=np.float32).reshape(C, UNITS, FW)
    rx = np.asarray(inputs["rx"]).astype(np.int64)
    ry = np.asarray(inputs["ry"]).astype(np.int64)

    u = np.arange(RF)
    ix = rx[:, None] + u                     # [GX, RF]
    iy = ry[:, None] + u                     # [GY, RF]
    px = x[:, ix, :]                         # [C, GX, RF, IMG]
    patches = px[:, :, :, iy]                # [C, GX, RF, GY, RF]
    patches = np.ascontiguousarray(patches.transpose(0, 1, 3, 2, 4))
    patches = patches.reshape(C, UNITS, FA)

    qd, sd = _quant_row(we - wi)
    wab = wa.astype(ml_dtypes.bfloat16).view(np.int8).reshape(C, UNITS, 2 * FA)
    pab = patches.astype(ml_dtypes.bfloat16).view(np.int8).reshape(C, UNITS, 2 * FA)
    blk = np.concatenate([qd, wab, pab], axis=2)     # [C, UNITS, UB] bytes
    possb_all = GAMMA * prev * sd                    # [C, UNITS]

    selm = (np.arange(128)[:, None] % S == np.arange(S)[None, :]).astype(np.float32)

    in_maps = []
    for k in range(N_CORES):
        n0 = k * PER_CORE
        b = np.zeros((C, PAD, UB), np.int8)
        b[:, :PER_CORE] = blk[:, n0:n0 + PER_CORE]
        pb = np.zeros((C, PAD), np.float32)
        pb[:, :PER_CORE] = possb_all[:, n0:n0 + PER_CORE]

        blob = b.reshape(C, NG, S, UB).transpose(0, 2, 1, 3).reshape(128, NG * UB)
        cst = np.zeros((128, 2 * NG + S), np.float32)
        cst[:, 0:NG] = pb.reshape(C, NG, S).transpose(0, 2, 1).reshape(128, NG)
        cst[:, NG:2 * NG] = 1.0                      # bf16 afferent: unit scale
        cst[:, 2 * NG:] = selm
        in_maps.append({
            "blob": np.ascontiguousarray(blob),
            "consts": cst,
        })
    return in_maps


def _assemble_output(results):
    act = np.empty(UNITS, np.float32)
    for k in range(N_CORES):
        o = np.asarray(results[k]["out"])            # [S, NG]
        loc = o.T.reshape(PAD)                       # unit n_local = 8g + s
        act[k * PER_CORE:(k + 1) * PER_CORE] = loc[:PER_CORE]
    out = np.broadcast_to(act.reshape(1, GX, GY), (C, GX, GY))
    return np.ascontiguousarray(out, dtype=np.float32)


def kernel(**inputs):
    nc = _get_program()
    in_maps = _prep_in_maps(inputs)
    res = run_bass_kernel_spmd(nc, in_maps, core_ids=list(range(N_CORES)))
    return _assemble_output(res.results)


# revision 15
# speedup vs baseline: 1.0702x; 1.0157x over previous
"""Trainium2 Bass kernel for nn_CortexNetwork (dense_cnn, memory-bound).

Reference computation:
    patches[c,i,j,u,v] = x[c, rx[i]+u, ry[j]+v]
    aff[i,j] = sum_{c,u,v} patches * Wa
    exc[i,j] = sum_c prev[c,i,j] * sum_{x,y} We[c,i,j,x,y]   (inh likewise, Wi)
    out      = broadcast_c(relu(aff + 0.9*exc - 0.9*inh))

Strategy: tensor-parallel over the 36x36=1296 grid units, 162 units per
core (padded to 168 = 21 groups of 8 so every DMA covers the full 128
partitions; partition = c*8+s).  The output depends on the lateral
weights only through D = We - Wi (same prev multiplier, same gamma), so
the host ships D instead of both tensors, quantized to int8 with
per-(c,unit)-row absmax scales that fold into the per-partition
possb = 0.9*prev*sD; Wa and the gathered patches ship as bf16 so the
fused afferent multiply runs in the DVE 2x packed mode.  Exact offline
rel-err of this scheme on the true inputs is 0.0070 vs the 2e-2 gate.
Per unit the device streams 1296B (D int8) + 2304B (Wa|patch bf16)
= 3600B -> 9.7MB/core, moved by 6 large column-slice DMAs from one
DRAM blob (few tensors keep the NEFF preamble TENSOR_LOADs short).

Per DMA batch (gcnt groups): one 3D bf16 multiply and one batched
tensor_reduce on DVE produce the afferent partials; the lateral reduce
is one scaled reduction per group, interleaved over ScalarE
(activation Copy + scale + accum_out, 17 groups) and DVE
(tensor_scalar mult + accum_out, 4 groups) so both engines drain
concurrently with the DMA stream.  Channel sums are 0/1-selector
matmuls on the idle PE, then relu.
"""

import numpy as np
import ml_dtypes

import concourse.bass as bass
import concourse.bacc as bacc
import concourse.mybir as mybir
from concourse import tile
from concourse.bass_utils import run_bass_kernel_spmd

N_CORES = 8
C = 16
GX = GY = 36
RF = 24
IMG = 64
GAMMA = 0.9

UNITS = GX * GY                  # 1296
PER_CORE = UNITS // N_CORES      # 162
S = 8                            # units per group (partition dim C*S=128)
NG = 21                          # groups per core (168 units, 6 padded)
PAD = NG * S                     # 168
FW = GX * GY                     # lateral cols per unit: 1296
FA = RF * RF                     # afferent cols per unit: 576
UB = FW + 4 * FA                 # bytes per unit: 1296 + 2304 = 3600
UH = UB // 2                     # bf16 elements per unit view: 1800

DMA_G = [1, 2, 4, 4, 4, 4, 2]
DMA_START = np.concatenate([[0], np.cumsum(DMA_G)]).tolist()
DVE_D_GROUPS = (4, 9, 14, 19)    # lateral reduce on DVE; rest on ScalarE

_PROGRAM_CACHE = {}


def _build_program():
    f32 = mybir.dt.float32
    i8 = mybir.dt.int8
    bf16 = mybir.dt.bfloat16
    AL = mybir.AluOpType
    AF = mybir.ActivationFunctionType

    nc = bacc.Bacc(
        "TRN2", target_bir_lowering=False, debug=False, num_devices=N_CORES
    )
    blob_d = nc.dram_tensor("blob", [128, NG * UB], i8, kind="ExternalInput").ap()
    # consts: possb[0:21] | sap[21:42] | sel[42:50]
    consts_d = nc.dram_tensor("consts", [128, 2 * NG + S], f32, kind="ExternalInput").ap()
    out_d = nc.dram_tensor("out", [S, NG], f32, kind="ExternalOutput").ap()

    with tile.TileContext(nc) as tc:
        with (
            tc.tile_pool(name="w", bufs=1) as wp,
            tc.tile_pool(name="cst", bufs=1) as cp,
            tc.tile_pool(name="junk", bufs=1) as jp,
            tc.tile_pool(name="fin", bufs=1) as fp,
            tc.tile_pool(name="ps", bufs=1, space="PSUM") as pp,
        ):
            consts = cp.tile([128, 2 * NG + S], f32, tag="consts")
            plat = cp.tile([128, NG], f32, tag="plat")
            paffr = cp.tile([128, NG], f32, tag="paffr")
            warm = cp.tile([128, 1], f32, tag="warm")
            nc.sync.dma_start(consts[:], consts_d[:])

            wtiles = []
            for i, gcnt in enumerate(DMA_G):
                g0 = DMA_START[i]
                w = wp.tile([128, gcnt, UB], i8, tag=f"w{i}", name=f"w{i}")
                nc.sync.dma_start(w[:], blob_d[:, g0 * UB:(g0 + gcnt) * UB])
                wtiles.append(w)

            # warm the ACT spline table before the stream lands so the
            # first real activation doesn't pay the table load
            nc.scalar.activation(warm[:], consts[:, 0:1], AF.Copy)

            js = jp.tile([128, FW], bf16, tag="js")
            jv = jp.tile([128, FW], bf16, tag="jv")

            for i, gcnt in enumerate(DMA_G):
                g0 = DMA_START[i]
                w = wtiles[i]
                wb = w[:].bitcast(bf16)          # [128, gcnt, UH]
                jprod = jp.tile([128, gcnt, FA], bf16, tag=f"jp{i}", name=f"jprod{i}")
                nc.vector.tensor_mul(
                    jprod[:], wb[:, :, 648:648 + FA], wb[:, :, 648 + FA:UH]
                )
                nc.vector.tensor_reduce(
                    paffr[:, g0:g0 + gcnt], jprod[:],
                    axis=mybir.AxisListType.X, op=AL.add,
                )
                for gl in range(gcnt):
                    g = g0 + gl
                    dv = w[:, gl, 0:FW]
                    possb_col = consts[:, g:g + 1]
                    if g in DVE_D_GROUPS:
                        nc.vector.tensor_scalar(
                            jv[:], dv, possb_col, 0.0, AL.mult, AL.add,
                            accum_out=plat[:, g:g + 1],
                        )
                    else:
                        nc.scalar.activation(
                            js[:], dv, AF.Copy, scale=possb_col,
                            accum_out=plat[:, g:g + 1],
                        )

            # channel sums on PE (bf16 afferent needs no row scales)
            sel = consts[:, 2 * NG:2 * NG + S]
            psum = pp.tile([S, NG], f32, tag="ps")
            nc.tensor.matmul(psum[:], sel, plat[:], start=True, stop=False)
            nc.tensor.matmul(psum[:], sel, paffr[:], start=False, stop=True)

            res = fp.tile([S, NG], f32, tag="res")
            nc.vector.tensor_scalar_max(res[:], psum[:], 0.0)
            nc.sync.dma_start(out_d[:], res[:])

    nc.compile()
    return nc


def _get_program():
    if "nc" not in _PROGRAM_CACHE:
        _PROGRAM_CACHE["nc"] = _build_program()
    return _PROGRAM_CACHE["nc"]


def _quant_row(a):
    """Per-(c,row) symmetric int8 quantization of [C, N, K] -> int8, scale[C,N]."""
    s = np.abs(a).max(axis=2) / 127.0
    s = np.maximum(s, 1e-30)
    q = np.clip(np.round(a / s[:, :, None]), -127, 127).astype(np.int8)
    return q, s


def _prep_in_maps(inputs):
    x = np.asarray(inputs["x"], dtype=np.float32)
    prev = np.asarray(inputs["prev_activity"], dtype=np.float32).reshape(C, UNITS)
    wa = np.asarray(inputs["afferent_weights"], dtype=np.float32).reshape(C, UNITS, FA)
    we = np.asarray(inputs["ex_lateral_weights"], dtype=np.float32).reshape(C, UNITS, FW)
    wi = np.asarray(inputs["in_lateral_weights"], dtype=np.float32).reshape(C, UNITS, FW)
    rx = np.asarray(inputs["rx"]).astype(np.int64)
    ry = np.asarray(inputs["ry"]).astype(np.int64)

    u = np.arange(RF)
    ix = rx[:, None] + u                     # [GX, RF]
    iy = ry[:, None] + u                     # [GY, RF]
    px = x[:, ix, :]                         # [C, GX, RF, IMG]
    patches = px[:, :, :, iy]                # [C, GX, RF, GY, RF]
    patches = np.ascontiguousarray(patches.transpose(0, 1, 3, 2, 4))
    patches = patches.reshape(C, UNITS, FA)

    qd, sd = _quant_row(we - wi)
    wab = wa.astype(ml_dtypes.bfloat16).view(np.int8).reshape(C, UNITS, 2 * FA)
    pab = patches.astype(ml_dtypes.bfloat16).view(np.int8).reshape(C, UNITS, 2 * FA)
    blk = np.concatenate([qd, wab, pab], axis=2)     # [C, UNITS, UB] bytes
    possb_all = GAMMA * prev * sd                    # [C, UNITS]

    selm = (np.arange(128)[:, None] % S == np.arange(S)[None, :]).astype(np.float32)

    in_maps = []
    for k in range(N_CORES):
        n0 = k * PER_CORE
        b = np.zeros((C, PAD, UB), np.int8)
        b[:, :PER_CORE] = blk[:, n0:n0 + PER_CORE]
        pb = np.zeros((C, PAD), np.float32)
        pb[:, :PER_CORE] = possb_all[:, n0:n0 + PER_CORE]

        blob = b.reshape(C, NG, S, UB).transpose(0, 2, 1, 3).reshape(128, NG * UB)
        cst = np.zeros((128, 2 * NG + S), np.float32)
        cst[:, 0:NG] = pb.reshape(C, NG, S).transpose(0, 2, 1).reshape(128, NG)
        cst[:, NG:2 * NG] = 1.0                      # bf16 afferent: unit scale
        cst[:, 2 * NG:] = selm
        in_maps.append({
            "blob": np.ascontiguousarray(blob),
            "consts": cst,
        })
    return in_maps


def _assemble_output(results):
    act = np.empty(UNITS, np.float32)
    for k in range(N_CORES):
        o = np.asarray(results[k]["out"])            # [S, NG]
        loc = o.T.reshape(PAD)                       # unit n_local = 8g + s
        act[k * PER_CORE:(k + 1) * PER_CORE] = loc[:PER_CORE]
    out = np.broadcast_to(act.reshape(1, GX, GY), (C, GX, GY))
    return np.ascontiguousarray(out, dtype=np.float32)


def kernel(**inputs):
    nc = _get_program()
    in_maps = _prep_in_maps(inputs)
    res = run_bass_kernel_spmd(nc, in_maps, core_ids=list(range(N_CORES)))
    return _assemble_output(res.results)
